# revision 1
# baseline (speedup 1.0000x reference)
"""D-MPNN encoder layer on 8 Trainium2 NeuronCores (Bass/Tile).

Sharding strategy
-----------------
Edge pairs are partitioned across 8 cores (50k pairs -> 100k directed edges per
core) and the node space is split into two halves (NH=25088) so every gather
table has < 32768 rows (int16 dma_gather indices).

Per core, edges are grouped into 4 classes by (src-half, dst-half), ordered so
that the reverse edge of class (s,d) slot i is class (d,s) slot i (off-diag)
or slot i +/- F (diagonal): the h[rev] read is a sequential xbar-transposed
DMA, never a gather.

Per message layer (fp16 storage, fp32 PSUM accumulation):
  m = dma_gather(tmp_half[src]) - h_prev[rev]        (both edge-major)
  h = relu(W_h @ m) via PE transpose of each 128-edge chunk + stationary
      matmuls -> edge-major f16
(No DMA-xbar transposes anywhere: concurrent xbar use by transposed gathers
and dma_start(transpose=True) corrupts data on this hardware.)
Segment-sum: per dst-window (128 nodes), edge rows are re-gathered and reduced
with DVE-generated one-hot selection matmuls accumulating in PSUM; per-core
partials are AllReduced (layers 0,1) / ReduceScattered (layer 2, so each core
holds exactly its output node slice).

Layer 0 gathers proj_atom = atom @ W_i_atom.T (dense, computed on-device) and
adds a streaming W_i_bond matmul. The final layer is a dense feature-major
matmul per node slice; the host transposes/concatenates the 8 slices.
"""

import sys
import numpy as np

sys.path.insert(0, "/opt/trn_rl_repo")

# ---------------------------------------------------------------- constants
N_NODES = 50000
N_PAIRS = 400000
ATOM_FDIM = 133
BOND_FDIM = 14
HIDDEN = 128
DEPTH = 3
N_CORES = 8
NH = 25088                      # node half size (196 windows of 128)

GOP = 4096                      # rows per dma_gather instruction
BLK = 512                       # matmul free-dim block

F16 = np.float16
I16 = np.int16

CLS_NAMES = ["00", "01", "10", "11"]
DEBUG_DUMPS = []
TRUNC = DEPTH  # build only the first TRUNC layers (diagnostics)


def _derived():
    npad = 2 * NH
    return npad, NH // 128, npad // N_CORES


def _wrap_idx(idx):
    """int16 index array -> dma_gather SBUF layout [128, n/16]."""
    n = idx.shape[0]
    assert n % 16 == 0
    return np.tile(idx.reshape(n // 16, 16).T, (8, 1)).copy()


def _ceil_to(x, m):
    return ((x + m - 1) // m) * m


def _host_prep(atom_feats, bond_feats, W_i, W_h, W_o, src, dst):
    NPAD, NWIN, OUT_SLICE = _derived()
    src = np.asarray(src).astype(np.int64)
    dst = np.asarray(dst).astype(np.int64)
    u = src[:N_PAIRS]
    v = dst[:N_PAIRS]
    ppc = N_PAIRS // N_CORES

    per_core = []
    for c in range(N_CORES):
        pu = u[c * ppc:(c + 1) * ppc]
        pv = v[c * ppc:(c + 1) * ppc]
        gp = np.arange(c * ppc, (c + 1) * ppc, dtype=np.int64)
        per_core.append((pu, pv, gp, (pu >= NH).astype(np.int8),
                         (pv >= NH).astype(np.int8)))

    n00 = max(int(((p[3] == 0) & (p[4] == 0)).sum()) for p in per_core)
    n11 = max(int(((p[3] == 1) & (p[4] == 1)).sum()) for p in per_core)
    n01 = max(int(((p[3] != p[4])).sum()) for p in per_core)
    F00 = _ceil_to(max(n00, 512), 1024)
    F11 = _ceil_to(max(n11, 512), 1024)
    S01 = _ceil_to(max(n01, 1024), 1024)
    sizes = {"00": 2 * F00, "01": S01, "10": S01, "11": 2 * F11}
    cls_off = {}
    off = 0
    for cn in CLS_NAMES:
        cls_off[cn] = off
        off += sizes[cn]
    S_TOT = off

    def pad_to(arr, size, fill):
        out = np.full(size, fill, dtype=np.int64)
        out[:arr.shape[0]] = arr
        return out

    maps = []
    for c in range(N_CORES):
        pu, pv, gp, ha, hb = per_core[c]
        cls = {}
        for hh, F in ((0, F00), (1, F11)):
            m = (ha == hh) & (hb == hh)
            fu, fv, fg = pu[m], pv[m], gp[m]
            nr = fu.shape[0]
            real = np.zeros(2 * F, dtype=bool)
            real[:nr] = True
            real[F:F + nr] = True
            cls[f"{hh}{hh}"] = (
                np.concatenate([pad_to(fu, F, hh * NH), pad_to(fv, F, hh * NH)]),
                np.concatenate([pad_to(fv, F, hh * NH), pad_to(fu, F, hh * NH)]),
                np.concatenate([pad_to(fg, F, -1), pad_to(fg + N_PAIRS, F, -1)]),
                real)
        m01 = (ha == 0) & (hb == 1)
        m10 = (ha == 1) & (hb == 0)
        au, av, ag = pu[m01], pv[m01], gp[m01]
        bu, bv, bg = pu[m10], pv[m10], gp[m10]
        nr = au.shape[0] + bu.shape[0]
        real = np.zeros(S01, dtype=bool)
        real[:nr] = True
        cls["01"] = (pad_to(np.concatenate([au, bv]), S01, 0),
                     pad_to(np.concatenate([av, bu]), S01, NH),
                     pad_to(np.concatenate([ag, bg + N_PAIRS]), S01, -1), real)
        cls["10"] = (pad_to(np.concatenate([av, bu]), S01, NH),
                     pad_to(np.concatenate([au, bv]), S01, 0),
                     pad_to(np.concatenate([ag + N_PAIRS, bg]), S01, -1),
                     real.copy())
        maps.append(cls)

    # seg-sum chunk count C
    C = 1
    for c in range(N_CORES):
        for cn in CLS_NAMES:
            s_arr, d_arr, e_arr, real = maps[c][cn]
            dl = (d_arr - int(cn[1]) * NH)[real]
            cnt = np.bincount(dl // 128, minlength=NWIN)
            if cnt.size:
                C = max(C, int(np.ceil(cnt.max() / 128)))
    SEG = NWIN * C * 128

    meta = dict(F00=F00, F11=F11, S01=S01, sizes=sizes, cls_off=cls_off,
                S_TOT=S_TOT, C=C, SEG=SEG)

    bond_feats = np.asarray(bond_feats, dtype=np.float32)
    atom_pad = np.zeros((NPAD, ATOM_FDIM), dtype=np.float32)
    atom_pad[:N_NODES] = np.asarray(atom_feats, dtype=np.float32)
    atomT = np.ascontiguousarray(atom_pad.T).astype(F16)

    W_i = np.asarray(W_i, dtype=np.float32)
    W_h = np.asarray(W_h, dtype=np.float32)
    W_o = np.asarray(W_o, dtype=np.float32)
    wiaT = np.ascontiguousarray(W_i[:, :ATOM_FDIM].T).astype(F16)
    wibT = np.ascontiguousarray(W_i[:, ATOM_FDIM:].T).astype(F16)
    whT = np.ascontiguousarray(W_h.T).astype(F16)
    woaT = np.ascontiguousarray(W_o[:, :ATOM_FDIM].T).astype(F16)
    womT = np.ascontiguousarray(W_o[:, ATOM_FDIM:].T).astype(F16)

    iotaf = np.tile(np.arange(128, dtype=F16)[None, :], (128, 4)).copy()
    ident = np.eye(128, dtype=F16)

    shared = {
        "atomT_a": atomT[:128], "atomT_b": atomT[128:ATOM_FDIM],
        "wia_a": wiaT[:128], "wia_b": wiaT[128:ATOM_FDIM], "wib": wibT,
        "wh": whT, "woa_a": woaT[:128], "woa_b": woaT[128:ATOM_FDIM],
        "wom": womT, "iotaf": iotaf, "ident": ident,
    }

    in_maps = []
    for c in range(N_CORES):
        cls = maps[c]
        src16_all = np.zeros(S_TOT, dtype=I16)
        bondT = np.zeros((BOND_FDIM, S_TOT), dtype=F16)
        seg16 = np.zeros(4 * SEG, dtype=I16)
        dstl = np.full((128, 4 * NWIN * C), 200.0, dtype=F16)
        for ci, cn in enumerate(CLS_NAMES):
            s_half, d_half = int(cn[0]), int(cn[1])
            o, sz = cls_off[cn], sizes[cn]
            s_arr, d_arr, e_arr, real = cls[cn]
            src16_all[o:o + sz] = (s_arr - s_half * NH).astype(I16)
            realm = e_arr >= 0
            cols = np.zeros((BOND_FDIM, sz), dtype=F16)
            cols[:, realm] = bond_feats[e_arr[realm]].T.astype(F16)
            bondT[:, o:o + sz] = cols

            dl_all = d_arr - d_half * NH
            slots = np.nonzero(real)[0]
            dl = dl_all[real]
            w = dl // 128
            order = np.argsort(w, kind="stable")
            slots_s, dl_s, w_s = slots[order], dl[order], w[order]
            cnt = np.bincount(w_s, minlength=NWIN)
            starts = np.zeros(NWIN + 1, dtype=np.int64)
            np.cumsum(cnt, out=starts[1:])
            out_idx = np.zeros(SEG, dtype=I16)
            out_dl = np.full(SEG, 200.0, dtype=np.float32)
            for wi in range(NWIN):
                a, b = starts[wi], starts[wi + 1]
                base = wi * C * 128
                out_idx[base:base + (b - a)] = slots_s[a:b].astype(I16)
                out_dl[base:base + (b - a)] = (dl_s[a:b] - wi * 128).astype(
                    np.float32)
            seg16[ci * SEG:(ci + 1) * SEG] = out_idx
            dstl[:, ci * NWIN * C:(ci + 1) * NWIN * C] = (
                out_dl.reshape(NWIN * C, 128).T.astype(F16))

        sl = slice(c * OUT_SLICE, (c + 1) * OUT_SLICE)
        m = dict(shared)
        m["bondT"] = bondT
        m["src16w"] = _wrap_idx(src16_all)
        m["seg16w"] = _wrap_idx(seg16)
        m["dstl"] = dstl
        m["atomS_a"] = np.ascontiguousarray(atomT[:128, sl])
        m["atomS_b"] = np.ascontiguousarray(atomT[128:ATOM_FDIM, sl])
        in_maps.append(m)

    return meta, in_maps


# ------------------------------------------------------------------ program
def _build_program(meta):
    import concourse.bacc as bacc
    import concourse.tile as tile
    import concourse.mybir as mybir
    from concourse import library_config

    NPAD, NWIN, OUT_SLICE = _derived()
    f16, f32, i16 = mybir.dt.float16, mybir.dt.float32, mybir.dt.int16
    Relu = mybir.ActivationFunctionType.Relu

    sizes, cls_off = meta["sizes"], meta["cls_off"]
    S_TOT, C, SEG = meta["S_TOT"], meta["C"], meta["SEG"]
    F00, F11 = meta["F00"], meta["F11"]

    nc = bacc.Bacc("TRN2", target_bir_lowering=False, debug=False,
                   enable_asserts=False, num_devices=N_CORES,
                   num_swdge_queues=4)

    def din(name, shape, dt=f16):
        return nc.dram_tensor(name, shape, dt, kind="ExternalInput").ap()

    atomT_a = din("atomT_a", [128, NPAD])
    atomT_b = din("atomT_b", [ATOM_FDIM - 128, NPAD])
    atomS_a = din("atomS_a", [128, OUT_SLICE])
    atomS_b = din("atomS_b", [ATOM_FDIM - 128, OUT_SLICE])
    wia_a = din("wia_a", [128, HIDDEN])
    wia_b = din("wia_b", [ATOM_FDIM - 128, HIDDEN])
    wib = din("wib", [BOND_FDIM, HIDDEN])
    wh_t = din("wh", [HIDDEN, HIDDEN])
    woa_a = din("woa_a", [128, HIDDEN])
    woa_b = din("woa_b", [ATOM_FDIM - 128, HIDDEN])
    wom = din("wom", [HIDDEN, HIDDEN])
    iotaf = din("iotaf", [128, 512])
    ident_t = din("ident", [128, 128])
    bondT = din("bondT", [BOND_FDIM, S_TOT])
    src16w = din("src16w", [128, S_TOT // 16], i16)
    seg16w = din("seg16w", [128, 4 * SEG // 16], i16)
    dstl_t = din("dstl", [128, 4 * NWIN * C])

    out_t = nc.dram_tensor("out", [HIDDEN, OUT_SLICE], f32,
                           kind="ExternalOutput").ap()

    proj = nc.dram_tensor("proj", [NPAD, HIDDEN], f16, kind="Internal").ap()
    h_cls = {}
    for ell in range(DEPTH):
        for cn in CLS_NAMES:
            h_cls[(ell, cn)] = nc.dram_tensor(
                f"h{ell}_{cn}", [sizes[cn], HIDDEN], f16, kind="Internal").ap()
    partials = [nc.dram_tensor(f"partials{ell}", [NPAD, HIDDEN], f16,
                               kind="Internal").ap() for ell in range(DEPTH)]
    tmp = [nc.dram_tensor(f"tmp{ell}", [NPAD, HIDDEN], f16, kind="Internal",
                          addr_space="Shared").ap() for ell in range(DEPTH - 1)]
    rs_out = nc.dram_tensor("rsout", [OUT_SLICE, HIDDEN], f16,
                            kind="Internal").ap()

    nc.gpsimd.load_library(library_config.mlp)

    def rev_row(cn, slot):
        if cn == "01":
            return "10", slot
        if cn == "10":
            return "01", slot
        F = F00 if cn == "00" else F11
        return cn, (slot + F) if slot < F else (slot - F)

    cls_of_d = {0: ["00", "10"], 1: ["01", "11"]}

    with tile.TileContext(nc) as tc:
        with (
            tc.tile_pool(name="pers", bufs=1) as pers,
            tc.tile_pool(name="work", bufs=2) as work,
            tc.tile_pool(name="segw", bufs=2) as segw,
            tc.tile_pool(name="psum", bufs=2, space="PSUM") as psum,
            tc.tile_pool(name="psum1", bufs=2, space="PSUM") as psum1,
            tc.tile_pool(name="psum2", bufs=4, space="PSUM") as psum2,
        ):
            # ---------- persistent SBUF
            def pload(ap_in, shape, tag, dt=f16, eng="sync"):
                t = pers.tile(shape, dt, tag=tag)
                (nc.sync if eng == "sync" else nc.gpsimd).dma_start(t[:], ap_in)
                return t

            w_wh = pload(wh_t[:], [HIDDEN, HIDDEN], "w_wh")
            w_wib = pload(wib[:], [BOND_FDIM, HIDDEN], "w_wib")
            w_wia_a = pload(wia_a[:], [128, HIDDEN], "w_wia_a")
            w_wia_b = pload(wia_b[:], [ATOM_FDIM - 128, HIDDEN], "w_wia_b")
            w_woa_a = pload(woa_a[:], [128, HIDDEN], "w_woa_a")
            w_woa_b = pload(woa_b[:], [ATOM_FDIM - 128, HIDDEN], "w_woa_b")
            w_wom = pload(wom[:], [HIDDEN, HIDDEN], "w_wom")
            io_t = pload(iotaf[:], [128, 512], "io_t")
            sidx = pload(src16w[:], [128, S_TOT // 16], "sidx", i16)
            gidx = pload(seg16w[:], [128, 4 * SEG // 16], "gidx", i16)
            dstl_s = pload(dstl_t[:], [128, 4 * NWIN * C], "dstl_s")
            ident_s = pload(ident_t[:], [128, 128], "ident_s")

            # ---------- proj_atom [NPAD, H] f16 (node-major, edge gather table)
            for ch in range(NPAD // 128):
                a_t = work.tile([128, 128], f16, tag="pa")
                b_t = work.tile([ATOM_FDIM - 128, 128], f16, tag="pb")
                csl = slice(ch * 128, (ch + 1) * 128)
                nc.sync.dma_start(a_t[:], atomT_a[:, csl])
                nc.sync.dma_start(b_t[:], atomT_b[:, csl])
                ps = psum1.tile([128, 128], f32, tag="seg")
                nc.tensor.matmul(ps[:], lhsT=a_t[:], rhs=w_wia_a[:],
                                 start=True, stop=False)
                nc.tensor.matmul(ps[:], lhsT=b_t[:], rhs=w_wia_b[:],
                                 start=False, stop=True)
                o_t = work.tile([128, 128], f16, tag="po")
                nc.scalar.copy(o_t[:], ps[:])
                nc.sync.dma_start(
                    proj[csl, :].rearrange("(a p) d -> p a d", p=128),
                    o_t[:].unsqueeze(1))

            # ---------- layers
            for ell in range(min(DEPTH, TRUNC)):
                src_table = proj if ell == 0 else tmp[ell - 1]
                for cn in CLS_NAMES:
                    s_half = int(cn[0])
                    o, sz = cls_off[cn], sizes[cn]
                    table = src_table[s_half * NH:(s_half + 1) * NH, :]
                    pos = 0
                    while pos < sz:
                        g = min(GOP, sz - pos)
                        nblk = g // BLK
                        icols = sidx[:, (o + pos) // 16:(o + pos + g) // 16]
                        h_t = work.tile([128, (GOP // 128) * HIDDEN], f16,
                                        tag="ht")
                        if ell == 0:
                            g1 = work.tile([128, (GOP // 128) * HIDDEN], f16,
                                           tag="g1", bufs=3)
                            nc.gpsimd.dma_gather(
                                g1[:, :(g // 128) * HIDDEN].rearrange(
                                    "p (c d) -> p c d", d=HIDDEN),
                                table, icols, g, g, HIDDEN,
                                single_packet=False)
                            bt = work.tile([BOND_FDIM, GOP], f16, tag="bt")
                            nc.sync.dma_start(bt[:, :g],
                                              bondT[:, o + pos:o + pos + g])
                            for j in range(nblk):
                                ps = psum.tile([128, BLK], f32, tag="mm")
                                for q in range(4):
                                    ci = j * 4 + q
                                    nc.tensor.matmul(
                                        ps[:, q * 128:(q + 1) * 128],
                                        lhsT=bt[:, ci * 128:(ci + 1) * 128],
                                        rhs=w_wib[:], start=True, stop=True)
                                sl = slice(j * BLK, (j + 1) * BLK)
                                nc.vector.tensor_add(out=h_t[:, sl],
                                                     in0=g1[:, sl], in1=ps[:])
                                nc.scalar.activation(h_t[:, sl], h_t[:, sl],
                                                     Relu)
                        else:
                            # edge-major gather of tmp + edge-major rev read
                            g1 = work.tile([128, (GOP // 128) * HIDDEN], f16,
                                           tag="g1", bufs=3)
                            nc.gpsimd.dma_gather(
                                g1[:, :(g // 128) * HIDDEN].rearrange(
                                    "p (c d) -> p c d", d=HIDDEN),
                                table, icols, g, g, HIDDEN,
                                single_packet=False)
                            g2 = work.tile([128, (GOP // 128) * HIDDEN], f16,
                                           tag="g2", bufs=3)
                            for b1 in range(g // 1024):
                                rcn, rrow = rev_row(cn, pos + b1 * 1024)
                                nc.sync.dma_start(
                                    g2[:, b1 * 8 * HIDDEN:
                                       (b1 + 1) * 8 * HIDDEN].rearrange(
                                        "p (c d) -> p c d", d=HIDDEN),
                                    h_cls[(ell - 1, rcn)]
                                    [rrow:rrow + 1024, :].rearrange(
                                        "(c p) d -> p c d", p=128))
                            nc.vector.tensor_tensor(
                                out=g1[:, :(g // 128) * HIDDEN],
                                in0=g1[:, :(g // 128) * HIDDEN],
                                in1=g2[:, :(g // 128) * HIDDEN],
                                op=mybir.AluOpType.subtract)
                            for j in range(nblk):
                                ps = psum.tile([128, BLK], f32, tag="mm")
                                mt = work.tile([128, BLK], f16, tag="mt")
                                for q in range(4):
                                    ci = j * 4 + q
                                    tp = psum2.tile([128, 128], f16, tag="tp")
                                    nc.tensor.transpose(
                                        tp[:], g1[:, ci * 128:(ci + 1) * 128],
                                        ident_s[:])
                                    msl = slice(q * 128, (q + 1) * 128)
                                    nc.scalar.copy(mt[:, msl], tp[:])
                                    nc.tensor.matmul(
                                        ps[:, q * 128:(q + 1) * 128],
                                        lhsT=mt[:, msl],
                                        rhs=w_wh[:], start=True, stop=True)
                                nc.scalar.activation(
                                    h_t[:, j * BLK:(j + 1) * BLK], ps[:], Relu)
                        nc.sync.dma_start(
                            h_cls[(ell, cn)][pos:pos + g, :].rearrange(
                                "(c p) d -> p c d", p=128),
                            h_t[:, :(g // 128) * HIDDEN].rearrange(
                                "p (c d) -> p c d", d=HIDDEN))
                        pos += g

                # ---------- segment sum -> partials[ell]
                cur = {cn: dict(tile=None, base=-1) for cn in CLS_NAMES}
                s4 = {cn: dict(tile=None, base=-1) for cn in CLS_NAMES}
                for d in (0, 1):
                    for w in range(NWIN):
                        ps = psum1.tile([128, HIDDEN], f32, tag="seg")
                        n_mm = 2 * C
                        k = 0
                        for cn in cls_of_d[d]:
                            ci = CLS_NAMES.index(cn)
                            for chunk in range(C):
                                sp = w * C + chunk
                                slot = sp * 128
                                gb = (slot // GOP) * GOP
                                if cur[cn]["base"] != gb:
                                    g = min(GOP, SEG - gb)
                                    t = segw.tile(
                                        [128, (GOP // 128) * HIDDEN], f16,
                                        tag=f"sg_{cn}")
                                    nc.gpsimd.dma_gather(
                                        t[:, :(g // 128) * HIDDEN].rearrange(
                                            "p (c d) -> p c d", d=HIDDEN),
                                        h_cls[(ell, cn)][:],
                                        gidx[:, (ci * SEG + gb) // 16:
                                             (ci * SEG + gb + g) // 16],
                                        g, g, HIDDEN, single_packet=False)
                                    cur[cn] = dict(tile=t, base=gb)
                                sb = (sp // 4) * 4
                                if s4[cn]["base"] != sb:
                                    st = segw.tile([128, 512], f16,
                                                   tag=f"oh_{cn}")
                                    n4 = min(4, NWIN * C - sb)
                                    dcol = ci * NWIN * C + sb
                                    nc.vector.tensor_tensor(
                                        out=st[:, :n4 * 128].rearrange(
                                            "p (c n) -> p c n", n=128),
                                        in0=io_t[:, :n4 * 128].rearrange(
                                            "p (c n) -> p c n", n=128),
                                        in1=dstl_s[:, dcol:dcol + n4]
                                        .to_broadcast([128, n4, 128]),
                                        op=mybir.AluOpType.is_equal)
                                    s4[cn] = dict(tile=st, base=sb)
                                cb = (slot - cur[cn]["base"]) // 128
                                sq = sp - s4[cn]["base"]
                                nc.tensor.matmul(
                                    ps[:],
                                    lhsT=s4[cn]["tile"][:, sq * 128:
                                                        (sq + 1) * 128],
                                    rhs=cur[cn]["tile"][:, cb * HIDDEN:
                                                        (cb + 1) * HIDDEN],
                                    start=(k == 0), stop=(k == n_mm - 1))
                                k += 1
                        p_t = segw.tile([128, HIDDEN], f16, tag="pt")
                        nc.scalar.copy(p_t[:], ps[:])
                        row = d * NH + w * 128
                        nc.sync.dma_start(
                            partials[ell][row:row + 128, :].rearrange(
                                "(a p) d -> p a d", p=128),
                            p_t[:].unsqueeze(1))

                # ---------- collective
                if ell < DEPTH - 1:
                    nc.gpsimd.collective_compute(
                        "AllReduce", mybir.AluOpType.add,
                        replica_groups=[list(range(N_CORES))],
                        ins=[partials[ell][:]], outs=[tmp[ell][:]])
                else:
                    nc.gpsimd.collective_compute(
                        "ReduceScatter", mybir.AluOpType.add,
                        replica_groups=[list(range(N_CORES))],
                        ins=[partials[ell][:]], outs=[rs_out[:]])

            # ---------- final: out.T = relu(WoA@atom.T + WoM@msg.T) [H, slice]
            for ch in range(OUT_SLICE // 128 if TRUNC >= DEPTH else 1):
                if TRUNC < DEPTH:
                    d_t = work.tile([128, 128], f16, tag="fa")
                    o_t = work.tile([128, 128], f32, tag="fo")
                    nc.sync.dma_start(d_t[:], atomS_a[:, 0:128])
                    nc.scalar.copy(o_t[:], d_t[:])
                    nc.sync.dma_start(out_t[:, 0:128], o_t[:])
                    continue
                csl = slice(ch * 128, (ch + 1) * 128)
                a_t = work.tile([128, 128], f16, tag="fa")
                b_t = work.tile([ATOM_FDIM - 128, 128], f16, tag="fb")
                m_t = work.tile([128, 128], f16, tag="fm")
                mraw = work.tile([128, 128], f16, tag="fmr")
                nc.sync.dma_start(a_t[:], atomS_a[:, csl])
                nc.sync.dma_start(b_t[:], atomS_b[:, csl])
                nc.sync.dma_start(mraw[:], rs_out[csl, :])
                tpf = psum2.tile([128, 128], f16, tag="tp")
                nc.tensor.transpose(tpf[:], mraw[:], ident_s[:])
                nc.scalar.copy(m_t[:], tpf[:])
                ps = psum1.tile([128, 128], f32, tag="seg")
                nc.tensor.matmul(ps[:], lhsT=w_woa_a[:], rhs=a_t[:],
                                 start=True, stop=False)
                nc.tensor.matmul(ps[:], lhsT=w_woa_b[:], rhs=b_t[:],
                                 start=False, stop=False)
                nc.tensor.matmul(ps[:], lhsT=w_wom[:], rhs=m_t[:],
                                 start=False, stop=True)
                o_t = work.tile([128, 128], f32, tag="fo")
                nc.scalar.activation(o_t[:], ps[:], Relu)
                nc.sync.dma_start(out_t[:, csl], o_t[:])

    if DEBUG_DUMPS:
        with tile.TileContext(nc) as tc2:
            dbg_map = {"proj": proj, "tmp0": tmp[0], "tmp1": tmp[1],
                       "partials0": partials[0], "partials1": partials[1],
                       "partials2": partials[2], "rsout": rs_out}
            for ell in range(DEPTH):
                for cn in CLS_NAMES:
                    dbg_map[f"h{ell}_{cn}"] = h_cls[(ell, cn)]
            for name in DEBUG_DUMPS:
                t = dbg_map[name]
                o = nc.dram_tensor("dbg_" + name, list(t.shape), f16,
                                   kind="ExternalOutput").ap()
                nc.sync.dma_start(o[:], t[:])

    # Tile assigns SWDGE completion sems round-robin (DMASW<i>_*); the HW
    # locks each sem to one SWDGE queue, so spread gathers across the 4
    # queues by their assigned sem index.
    import re
    for b in nc.main_func.blocks:
        for ins in b.instructions:
            if type(ins).__name__ == "InstDMAGatherAnt" and ins.sync_info:
                for upd in ins.sync_info.on_update:
                    mname = upd.ant_name or ""
                    m = re.match(r"DMASW(\d+)_", mname)
                    if m:
                        ins.queue_num = int(m.group(1)) % 4
                        break

    nc.compile()
    return nc


# -------------------------------------------------------------------- entry
_CACHE = {}


def kernel(atom_feats, bond_feats, W_i, W_h, W_o, src, dst, reverse_e):
    from concourse import bass_utils

    NPAD, NWIN, OUT_SLICE = _derived()

    rev = np.asarray(reverse_e).astype(np.int64)
    ar = np.arange(N_PAIRS, dtype=np.int64)
    assert np.array_equal(rev[:N_PAIRS], ar + N_PAIRS) and \
        np.array_equal(rev[N_PAIRS:], ar), "unexpected reverse_e structure"

    meta, in_maps = _host_prep(atom_feats, bond_feats, W_i, W_h, W_o, src, dst)

    key = (meta["S_TOT"], meta["C"], meta["F00"], meta["F11"], meta["S01"])
    if key not in _CACHE:
        _CACHE[key] = _build_program(meta)
    nc = _CACHE[key]

    res = bass_utils.run_bass_kernel_spmd(
        nc, in_maps, core_ids=list(range(N_CORES)))
    out = np.concatenate(
        [res.results[c]["out"].T for c in range(N_CORES)], axis=0)
    return np.ascontiguousarray(out[:N_NODES]).astype(np.float32)



# revision 7
# speedup vs baseline: 1.7175x; 1.7175x over previous
"""D-MPNN encoder layer on 8 Trainium2 NeuronCores (Bass/Tile).

Sharding strategy
-----------------
Edge pairs are partitioned across 8 cores (50k pairs -> 100k directed edges per
core) and the node space is split into two halves (NH=25088) so every gather
table has < 32768 rows (int16 dma_gather indices).

Per core, edges are grouped into 4 classes by (src-half, dst-half), ordered so
that the reverse edge of class (s,d) slot i is class (d,s) slot i (off-diag)
or slot i +/- F (diagonal): the h[rev] read is a sequential xbar-transposed
DMA, never a gather.

Per message layer (fp16 storage, fp32 PSUM accumulation):
  m = dma_gather(tmp_half[src]) - h_prev[rev]        (both edge-major)
  h = relu(W_h @ m) via PE transpose of each 128-edge chunk + stationary
      matmuls -> edge-major f16
(No DMA-xbar transposes anywhere: concurrent xbar use by transposed gathers
and dma_start(transpose=True) corrupts data on this hardware.)
Segment-sum: per dst-window (128 nodes), edge rows are re-gathered and reduced
with DVE-generated one-hot selection matmuls accumulating in PSUM; per-core
partials are AllReduced (layers 0,1) / ReduceScattered (layer 2, so each core
holds exactly its output node slice).

Layer 0 gathers proj_atom = atom @ W_i_atom.T (dense, computed on-device) and
adds a streaming W_i_bond matmul. The final layer is a dense feature-major
matmul per node slice; the host transposes/concatenates the 8 slices.
"""

import sys
import numpy as np

sys.path.insert(0, "/opt/trn_rl_repo")

# ---------------------------------------------------------------- constants
N_NODES = 50000
N_PAIRS = 400000
ATOM_FDIM = 133
BOND_FDIM = 14
HIDDEN = 128
DEPTH = 3
N_CORES = 8
NH = 25088                      # node half size (196 windows of 128)

GOP = 4096                      # rows per dma_gather instruction
BLK = 512                       # matmul free-dim block

F16 = np.float16
I16 = np.int16

CLS_NAMES = ["00", "01", "10", "11"]
DEBUG_DUMPS = []
TRUNC = DEPTH  # build only the first TRUNC layers (diagnostics)
ABLATE = frozenset()  # timing diagnostics: {"nocoll","noseg","nomsg","nol0","noproj"}


def _derived():
    npad = 2 * NH
    return npad, NH // 128, npad // N_CORES


def _wrap_idx(idx):
    """int16 index array -> dma_gather SBUF layout [128, n/16]."""
    n = idx.shape[0]
    assert n % 16 == 0
    return np.tile(idx.reshape(n // 16, 16).T, (8, 1)).copy()


def _ceil_to(x, m):
    return ((x + m - 1) // m) * m


def _host_prep(atom_feats, bond_feats, W_i, W_h, W_o, src, dst):
    NPAD, NWIN, OUT_SLICE = _derived()
    src = np.asarray(src).astype(np.int64)
    dst = np.asarray(dst).astype(np.int64)
    u = src[:N_PAIRS]
    v = dst[:N_PAIRS]
    ppc = N_PAIRS // N_CORES

    per_core = []
    for c in range(N_CORES):
        pu = u[c * ppc:(c + 1) * ppc]
        pv = v[c * ppc:(c + 1) * ppc]
        gp = np.arange(c * ppc, (c + 1) * ppc, dtype=np.int64)
        per_core.append((pu, pv, gp, (pu >= NH).astype(np.int8),
                         (pv >= NH).astype(np.int8)))

    n00 = max(int(((p[3] == 0) & (p[4] == 0)).sum()) for p in per_core)
    n11 = max(int(((p[3] == 1) & (p[4] == 1)).sum()) for p in per_core)
    n01 = max(int(((p[3] != p[4])).sum()) for p in per_core)
    F00 = _ceil_to(max(n00, 512), 1024)
    F11 = _ceil_to(max(n11, 512), 1024)
    S01 = _ceil_to(max(n01, 1024), 1024)
    sizes = {"00": 2 * F00, "01": S01, "10": S01, "11": 2 * F11}
    cls_off = {}
    off = 0
    for cn in CLS_NAMES:
        cls_off[cn] = off
        off += sizes[cn]
    S_TOT = off

    def pad_to(arr, size, fill):
        out = np.full(size, fill, dtype=np.int64)
        out[:arr.shape[0]] = arr
        return out

    maps = []
    for c in range(N_CORES):
        pu, pv, gp, ha, hb = per_core[c]
        cls = {}
        for hh, F in ((0, F00), (1, F11)):
            m = (ha == hh) & (hb == hh)
            fu, fv, fg = pu[m], pv[m], gp[m]
            nr = fu.shape[0]
            real = np.zeros(2 * F, dtype=bool)
            real[:nr] = True
            real[F:F + nr] = True
            cls[f"{hh}{hh}"] = (
                np.concatenate([pad_to(fu, F, hh * NH), pad_to(fv, F, hh * NH)]),
                np.concatenate([pad_to(fv, F, hh * NH), pad_to(fu, F, hh * NH)]),
                np.concatenate([pad_to(fg, F, -1), pad_to(fg + N_PAIRS, F, -1)]),
                real)
        m01 = (ha == 0) & (hb == 1)
        m10 = (ha == 1) & (hb == 0)
        au, av, ag = pu[m01], pv[m01], gp[m01]
        bu, bv, bg = pu[m10], pv[m10], gp[m10]
        nr = au.shape[0] + bu.shape[0]
        real = np.zeros(S01, dtype=bool)
        real[:nr] = True
        cls["01"] = (pad_to(np.concatenate([au, bv]), S01, 0),
                     pad_to(np.concatenate([av, bu]), S01, NH),
                     pad_to(np.concatenate([ag, bg + N_PAIRS]), S01, -1), real)
        cls["10"] = (pad_to(np.concatenate([av, bu]), S01, NH),
                     pad_to(np.concatenate([au, bv]), S01, 0),
                     pad_to(np.concatenate([ag + N_PAIRS, bg]), S01, -1),
                     real.copy())
        maps.append(cls)

    # seg-sum chunk count C
    C = 1
    for c in range(N_CORES):
        for cn in CLS_NAMES:
            s_arr, d_arr, e_arr, real = maps[c][cn]
            dl = (d_arr - int(cn[1]) * NH)[real]
            cnt = np.bincount(dl // 128, minlength=NWIN)
            if cnt.size:
                C = max(C, int(np.ceil(cnt.max() / 128)))
    SEG = NWIN * C * 128

    meta = dict(F00=F00, F11=F11, S01=S01, sizes=sizes, cls_off=cls_off,
                S_TOT=S_TOT, C=C, SEG=SEG)

    bond_feats = np.asarray(bond_feats, dtype=np.float32)
    atom_pad = np.zeros((NPAD, ATOM_FDIM), dtype=np.float32)
    atom_pad[:N_NODES] = np.asarray(atom_feats, dtype=np.float32)
    atomT = np.ascontiguousarray(atom_pad.T).astype(F16)

    W_i = np.asarray(W_i, dtype=np.float32)
    W_h = np.asarray(W_h, dtype=np.float32)
    W_o = np.asarray(W_o, dtype=np.float32)
    wiaT = np.ascontiguousarray(W_i[:, :ATOM_FDIM].T).astype(F16)
    wibT = np.ascontiguousarray(W_i[:, ATOM_FDIM:].T).astype(F16)
    whT = np.ascontiguousarray(W_h.T).astype(F16)
    woaT = np.ascontiguousarray(W_o[:, :ATOM_FDIM].T).astype(F16)
    womT = np.ascontiguousarray(W_o[:, ATOM_FDIM:].T).astype(F16)

    iotaf = np.tile(np.arange(128, dtype=F16)[None, :], (128, 4)).copy()
    ident = np.eye(128, dtype=F16)

    shared = {
        "atomT_a": atomT[:128], "atomT_b": atomT[128:ATOM_FDIM],
        "wia_a": wiaT[:128], "wia_b": wiaT[128:ATOM_FDIM], "wib": wibT,
        "wh": whT, "woa_a": woaT[:128], "woa_b": woaT[128:ATOM_FDIM],
        "wom": womT, "iotaf": iotaf, "ident": ident,
    }

    in_maps = []
    for c in range(N_CORES):
        cls = maps[c]
        src16_all = np.zeros(S_TOT, dtype=I16)
        bondT = np.zeros((BOND_FDIM, S_TOT), dtype=F16)
        seg16 = np.zeros(4 * SEG, dtype=I16)
        dstl = np.full((128, 4 * NWIN * C), 200.0, dtype=F16)
        for ci, cn in enumerate(CLS_NAMES):
            s_half, d_half = int(cn[0]), int(cn[1])
            o, sz = cls_off[cn], sizes[cn]
            s_arr, d_arr, e_arr, real = cls[cn]
            src16_all[o:o + sz] = (s_arr - s_half * NH).astype(I16)
            realm = e_arr >= 0
            cols = np.zeros((BOND_FDIM, sz), dtype=F16)
            cols[:, realm] = bond_feats[e_arr[realm]].T.astype(F16)
            bondT[:, o:o + sz] = cols

            dl_all = d_arr - d_half * NH
            slots = np.nonzero(real)[0]
            dl = dl_all[real]
            w = dl // 128
            order = np.argsort(w, kind="stable")
            slots_s, dl_s, w_s = slots[order], dl[order], w[order]
            cnt = np.bincount(w_s, minlength=NWIN)
            starts = np.zeros(NWIN + 1, dtype=np.int64)
            np.cumsum(cnt, out=starts[1:])
            out_idx = np.zeros(SEG, dtype=I16)
            out_dl = np.full(SEG, 200.0, dtype=np.float32)
            for wi in range(NWIN):
                a, b = starts[wi], starts[wi + 1]
                base = wi * C * 128
                out_idx[base:base + (b - a)] = slots_s[a:b].astype(I16)
                out_dl[base:base + (b - a)] = (dl_s[a:b] - wi * 128).astype(
                    np.float32)
            seg16[ci * SEG:(ci + 1) * SEG] = out_idx
            dstl[:, ci * NWIN * C:(ci + 1) * NWIN * C] = (
                out_dl.reshape(NWIN * C, 128).T.astype(F16))

        sl = slice(c * OUT_SLICE, (c + 1) * OUT_SLICE)
        m = dict(shared)
        m["bondT"] = bondT
        m["src16w"] = _wrap_idx(src16_all)
        m["seg16w"] = _wrap_idx(seg16)
        m["dstl"] = dstl
        m["atomS_a"] = np.ascontiguousarray(atomT[:128, sl])
        m["atomS_b"] = np.ascontiguousarray(atomT[128:ATOM_FDIM, sl])
        in_maps.append(m)

    return meta, in_maps


# ------------------------------------------------------------------ program
def _build_program(meta):
    import concourse.bacc as bacc
    import concourse.tile as tile
    import concourse.mybir as mybir
    from concourse import library_config

    NPAD, NWIN, OUT_SLICE = _derived()
    f16, f32, i16 = mybir.dt.float16, mybir.dt.float32, mybir.dt.int16
    Relu = mybir.ActivationFunctionType.Relu

    sizes, cls_off = meta["sizes"], meta["cls_off"]
    S_TOT, C, SEG = meta["S_TOT"], meta["C"], meta["SEG"]
    F00, F11 = meta["F00"], meta["F11"]

    nc = bacc.Bacc("TRN2", target_bir_lowering=False, debug=False,
                   enable_asserts=False, num_devices=N_CORES,
                   num_swdge_queues=4)

    def din(name, shape, dt=f16):
        return nc.dram_tensor(name, shape, dt, kind="ExternalInput").ap()

    atomT_a = din("atomT_a", [128, NPAD])
    atomT_b = din("atomT_b", [ATOM_FDIM - 128, NPAD])
    atomS_a = din("atomS_a", [128, OUT_SLICE])
    atomS_b = din("atomS_b", [ATOM_FDIM - 128, OUT_SLICE])
    wia_a = din("wia_a", [128, HIDDEN])
    wia_b = din("wia_b", [ATOM_FDIM - 128, HIDDEN])
    wib = din("wib", [BOND_FDIM, HIDDEN])
    wh_t = din("wh", [HIDDEN, HIDDEN])
    woa_a = din("woa_a", [128, HIDDEN])
    woa_b = din("woa_b", [ATOM_FDIM - 128, HIDDEN])
    wom = din("wom", [HIDDEN, HIDDEN])
    iotaf = din("iotaf", [128, 512])
    ident_t = din("ident", [128, 128])
    bondT = din("bondT", [BOND_FDIM, S_TOT])
    src16w = din("src16w", [128, S_TOT // 16], i16)
    seg16w = din("seg16w", [128, 4 * SEG // 16], i16)
    dstl_t = din("dstl", [128, 4 * NWIN * C])

    out_t = nc.dram_tensor("out", [HIDDEN, OUT_SLICE], f32,
                           kind="ExternalOutput").ap()

    proj = nc.dram_tensor("proj", [NPAD, HIDDEN], f16, kind="Internal").ap()
    h_cls = {}
    for ell in range(DEPTH):
        for cn in CLS_NAMES:
            h_cls[(ell, cn)] = nc.dram_tensor(
                f"h{ell}_{cn}", [sizes[cn], HIDDEN], f16, kind="Internal").ap()
    partials = [nc.dram_tensor(f"partials{ell}", [NPAD, HIDDEN], f16,
                               kind="Internal").ap() for ell in range(DEPTH)]
    tmp = [nc.dram_tensor(f"tmp{ell}", [NPAD, HIDDEN], f16, kind="Internal",
                          addr_space="Shared").ap() for ell in range(DEPTH - 1)]
    rs_out = nc.dram_tensor("rsout", [OUT_SLICE, HIDDEN], f16,
                            kind="Internal").ap()

    nc.gpsimd.load_library(library_config.mlp)

    def rev_row(cn, slot):
        if cn == "01":
            return "10", slot
        if cn == "10":
            return "01", slot
        F = F00 if cn == "00" else F11
        return cn, (slot + F) if slot < F else (slot - F)

    cls_of_d = {0: ["00", "10"], 1: ["01", "11"]}

    with tile.TileContext(nc) as tc:
        with (
            tc.tile_pool(name="pers", bufs=1) as pers,
            tc.tile_pool(name="work", bufs=2) as work,
            tc.tile_pool(name="segw", bufs=2) as segw,
            tc.tile_pool(name="psum", bufs=2, space="PSUM") as psum,
            tc.tile_pool(name="psum1", bufs=2, space="PSUM") as psum1,
            tc.tile_pool(name="psum2", bufs=4, space="PSUM") as psum2,
        ):
            # ---------- persistent SBUF
            def pload(ap_in, shape, tag, dt=f16, eng="sync"):
                t = pers.tile(shape, dt, tag=tag)
                (nc.sync if eng == "sync" else nc.gpsimd).dma_start(t[:], ap_in)
                return t

            w_wh = pload(wh_t[:], [HIDDEN, HIDDEN], "w_wh")
            w_wib = pload(wib[:], [BOND_FDIM, HIDDEN], "w_wib")
            w_wia_a = pload(wia_a[:], [128, HIDDEN], "w_wia_a")
            w_wia_b = pload(wia_b[:], [ATOM_FDIM - 128, HIDDEN], "w_wia_b")
            w_woa_a = pload(woa_a[:], [128, HIDDEN], "w_woa_a")
            w_woa_b = pload(woa_b[:], [ATOM_FDIM - 128, HIDDEN], "w_woa_b")
            w_wom = pload(wom[:], [HIDDEN, HIDDEN], "w_wom")
            io_t = pload(iotaf[:], [128, 512], "io_t")
            sidx = pload(src16w[:], [128, S_TOT // 16], "sidx", i16)
            gidx = pload(seg16w[:], [128, 4 * SEG // 16], "gidx", i16)
            dstl_s = pload(dstl_t[:], [128, 4 * NWIN * C], "dstl_s")
            ident_s = pload(ident_t[:], [128, 128], "ident_s")

            # ---------- proj_atom [NPAD, H] f16 (node-major, edge gather table)
            for ch in range(0 if "noproj" in ABLATE else NPAD // 128):
                a_t = work.tile([128, 128], f16, tag="pa")
                b_t = work.tile([ATOM_FDIM - 128, 128], f16, tag="pb")
                csl = slice(ch * 128, (ch + 1) * 128)
                nc.sync.dma_start(a_t[:], atomT_a[:, csl])
                nc.sync.dma_start(b_t[:], atomT_b[:, csl])
                ps = psum1.tile([128, 128], f32, tag="seg")
                nc.tensor.matmul(ps[:], lhsT=a_t[:], rhs=w_wia_a[:],
                                 start=True, stop=False)
                nc.tensor.matmul(ps[:], lhsT=b_t[:], rhs=w_wia_b[:],
                                 start=False, stop=True)
                o_t = work.tile([128, 128], f16, tag="po")
                nc.scalar.copy(o_t[:], ps[:])
                nc.sync.dma_start(
                    proj[csl, :].rearrange("(a p) d -> p a d", p=128),
                    o_t[:].unsqueeze(1))

            # ---------- layers
            for ell in range(min(DEPTH, TRUNC)):
                src_table = proj if ell == 0 else tmp[ell - 1]
                skip_edge = ("nol0" in ABLATE and ell == 0) or \
                    ("nomsg" in ABLATE and ell >= 1)
                for cn in (() if skip_edge else CLS_NAMES):
                    s_half = int(cn[0])
                    o, sz = cls_off[cn], sizes[cn]
                    table = src_table[s_half * NH:(s_half + 1) * NH, :]
                    pos = 0
                    while pos < sz:
                        g = min(GOP, sz - pos)
                        nblk = g // BLK
                        icols = sidx[:, (o + pos) // 16:(o + pos + g) // 16]
                        h_t = work.tile([128, (GOP // 128) * HIDDEN], f16,
                                        tag="ht")
                        if ell == 0:
                            g1 = work.tile([128, (GOP // 128) * HIDDEN], f16,
                                           tag="g1", bufs=3)
                            nc.gpsimd.dma_gather(
                                g1[:, :(g // 128) * HIDDEN].rearrange(
                                    "p (c d) -> p c d", d=HIDDEN),
                                table, icols, g, g, HIDDEN,
                                single_packet=False)
                            bt = work.tile([BOND_FDIM, GOP], f16, tag="bt")
                            nc.sync.dma_start(bt[:, :g],
                                              bondT[:, o + pos:o + pos + g])
                            for j in range(nblk):
                                ps = psum.tile([128, BLK], f32, tag="mm")
                                for q in range(4):
                                    ci = j * 4 + q
                                    nc.tensor.matmul(
                                        ps[:, q * 128:(q + 1) * 128],
                                        lhsT=bt[:, ci * 128:(ci + 1) * 128],
                                        rhs=w_wib[:], start=True, stop=True)
                                sl = slice(j * BLK, (j + 1) * BLK)
                                nc.vector.tensor_add(out=h_t[:, sl],
                                                     in0=g1[:, sl], in1=ps[:])
                                nc.scalar.activation(h_t[:, sl], h_t[:, sl],
                                                     Relu)
                        else:
                            # edge-major gather of tmp + edge-major rev read
                            g1 = work.tile([128, (GOP // 128) * HIDDEN], f16,
                                           tag="g1", bufs=3)
                            nc.gpsimd.dma_gather(
                                g1[:, :(g // 128) * HIDDEN].rearrange(
                                    "p (c d) -> p c d", d=HIDDEN),
                                table, icols, g, g, HIDDEN,
                                single_packet=False)
                            g2 = work.tile([128, (GOP // 128) * HIDDEN], f16,
                                           tag="g2", bufs=3)
                            for b1 in range(0 if "norev" in ABLATE
                                            else g // 1024):
                                rcn, rrow = rev_row(cn, pos + b1 * 1024)
                                nc.sync.dma_start(
                                    g2[:, b1 * 8 * HIDDEN:
                                       (b1 + 1) * 8 * HIDDEN].rearrange(
                                        "p (c d) -> p c d", d=HIDDEN),
                                    h_cls[(ell - 1, rcn)]
                                    [rrow:rrow + 1024, :].rearrange(
                                        "(c p) d -> p c d", p=128))
                            nc.vector.tensor_tensor(
                                out=g1[:, :(g // 128) * HIDDEN],
                                in0=g1[:, :(g // 128) * HIDDEN],
                                in1=g2[:, :(g // 128) * HIDDEN],
                                op=mybir.AluOpType.subtract)
                            for j in range(nblk):
                                ps = psum.tile([128, BLK], f32, tag="mm")
                                mt = work.tile([128, BLK], f16, tag="mt")
                                for q in range(4):
                                    ci = j * 4 + q
                                    tp = psum2.tile([128, 128], f16, tag="tp")
                                    nc.tensor.transpose(
                                        tp[:], g1[:, ci * 128:(ci + 1) * 128],
                                        ident_s[:])
                                    msl = slice(q * 128, (q + 1) * 128)
                                    nc.scalar.copy(mt[:, msl], tp[:])
                                    nc.tensor.matmul(
                                        ps[:, q * 128:(q + 1) * 128],
                                        lhsT=mt[:, msl],
                                        rhs=w_wh[:], start=True, stop=True)
                                nc.scalar.activation(
                                    h_t[:, j * BLK:(j + 1) * BLK], ps[:], Relu)
                        nc.sync.dma_start(
                            h_cls[(ell, cn)][pos:pos + g, :].rearrange(
                                "(c p) d -> p c d", p=128),
                            h_t[:, :(g // 128) * HIDDEN].rearrange(
                                "p (c d) -> p c d", d=HIDDEN))
                        pos += g

                # ---------- segment sum -> partials[ell]
                cur = {cn: dict(tile=None, base=-1) for cn in CLS_NAMES}
                s4 = {cn: dict(tile=None, base=-1) for cn in CLS_NAMES}
                for d in (() if "noseg" in ABLATE else (0, 1)):
                    for w in range(NWIN):
                        ps = psum1.tile([128, HIDDEN], f32, tag="seg")
                        n_mm = 2 * C
                        k = 0
                        for cn in cls_of_d[d]:
                            ci = CLS_NAMES.index(cn)
                            for chunk in range(C):
                                sp = w * C + chunk
                                slot = sp * 128
                                gb = (slot // GOP) * GOP
                                if cur[cn]["base"] != gb:
                                    g = min(GOP, SEG - gb)
                                    t = segw.tile(
                                        [128, (GOP // 128) * HIDDEN], f16,
                                        tag=f"sg_{cn}")
                                    nc.gpsimd.dma_gather(
                                        t[:, :(g // 128) * HIDDEN].rearrange(
                                            "p (c d) -> p c d", d=HIDDEN),
                                        h_cls[(ell, cn)][:],
                                        gidx[:, (ci * SEG + gb) // 16:
                                             (ci * SEG + gb + g) // 16],
                                        g, g, HIDDEN, single_packet=False)
                                    cur[cn] = dict(tile=t, base=gb)
                                sb = (sp // 4) * 4
                                if s4[cn]["base"] != sb:
                                    st = segw.tile([128, 512], f16,
                                                   tag=f"oh_{cn}")
                                    n4 = min(4, NWIN * C - sb)
                                    dcol = ci * NWIN * C + sb
                                    nc.vector.tensor_tensor(
                                        out=st[:, :n4 * 128].rearrange(
                                            "p (c n) -> p c n", n=128),
                                        in0=io_t[:, :n4 * 128].rearrange(
                                            "p (c n) -> p c n", n=128),
                                        in1=dstl_s[:, dcol:dcol + n4]
                                        .to_broadcast([128, n4, 128]),
                                        op=mybir.AluOpType.is_equal)
                                    s4[cn] = dict(tile=st, base=sb)
                                cb = (slot - cur[cn]["base"]) // 128
                                sq = sp - s4[cn]["base"]
                                nc.tensor.matmul(
                                    ps[:],
                                    lhsT=s4[cn]["tile"][:, sq * 128:
                                                        (sq + 1) * 128],
                                    rhs=cur[cn]["tile"][:, cb * HIDDEN:
                                                        (cb + 1) * HIDDEN],
                                    start=(k == 0), stop=(k == n_mm - 1))
                                k += 1
                        p_t = segw.tile([128, HIDDEN], f16, tag="pt")
                        nc.scalar.copy(p_t[:], ps[:])
                        row = d * NH + w * 128
                        nc.sync.dma_start(
                            partials[ell][row:row + 128, :].rearrange(
                                "(a p) d -> p a d", p=128),
                            p_t[:].unsqueeze(1))

                # ---------- collective
                if "nocoll" in ABLATE:
                    pass
                elif ell < DEPTH - 1:
                    nc.gpsimd.collective_compute(
                        "AllReduce", mybir.AluOpType.add,
                        replica_groups=[list(range(N_CORES))],
                        ins=[partials[ell][:]], outs=[tmp[ell][:]])
                else:
                    nc.gpsimd.collective_compute(
                        "ReduceScatter", mybir.AluOpType.add,
                        replica_groups=[list(range(N_CORES))],
                        ins=[partials[ell][:]], outs=[rs_out[:]])

            # ---------- final: out.T = relu(WoA@atom.T + WoM@msg.T) [H, slice]
            for ch in range(OUT_SLICE // 128 if TRUNC >= DEPTH else 1):
                if TRUNC < DEPTH:
                    d_t = work.tile([128, 128], f16, tag="fa")
                    o_t = work.tile([128, 128], f32, tag="fo")
                    nc.sync.dma_start(d_t[:], atomS_a[:, 0:128])
                    nc.scalar.copy(o_t[:], d_t[:])
                    nc.sync.dma_start(out_t[:, 0:128], o_t[:])
                    continue
                csl = slice(ch * 128, (ch + 1) * 128)
                a_t = work.tile([128, 128], f16, tag="fa")
                b_t = work.tile([ATOM_FDIM - 128, 128], f16, tag="fb")
                m_t = work.tile([128, 128], f16, tag="fm")
                mraw = work.tile([128, 128], f16, tag="fmr")
                nc.sync.dma_start(a_t[:], atomS_a[:, csl])
                nc.sync.dma_start(b_t[:], atomS_b[:, csl])
                nc.sync.dma_start(mraw[:], rs_out[csl, :])
                tpf = psum2.tile([128, 128], f16, tag="tp")
                nc.tensor.transpose(tpf[:], mraw[:], ident_s[:])
                nc.scalar.copy(m_t[:], tpf[:])
                ps = psum1.tile([128, 128], f32, tag="seg")
                nc.tensor.matmul(ps[:], lhsT=w_woa_a[:], rhs=a_t[:],
                                 start=True, stop=False)
                nc.tensor.matmul(ps[:], lhsT=w_woa_b[:], rhs=b_t[:],
                                 start=False, stop=False)
                nc.tensor.matmul(ps[:], lhsT=w_wom[:], rhs=m_t[:],
                                 start=False, stop=True)
                o_t = work.tile([128, 128], f32, tag="fo")
                nc.scalar.activation(o_t[:], ps[:], Relu)
                nc.sync.dma_start(out_t[:, csl], o_t[:])

    if DEBUG_DUMPS:
        with tile.TileContext(nc) as tc2:
            dbg_map = {"proj": proj, "tmp0": tmp[0], "tmp1": tmp[1],
                       "partials0": partials[0], "partials1": partials[1],
                       "partials2": partials[2], "rsout": rs_out}
            for ell in range(DEPTH):
                for cn in CLS_NAMES:
                    dbg_map[f"h{ell}_{cn}"] = h_cls[(ell, cn)]
            for name in DEBUG_DUMPS:
                t = dbg_map[name]
                o = nc.dram_tensor("dbg_" + name, list(t.shape), f16,
                                   kind="ExternalOutput").ap()
                nc.sync.dma_start(o[:], t[:])

    # Tile assigns SWDGE completion sems round-robin (DMASW<i>_*); the HW
    # locks each sem to one SWDGE queue, so spread gathers across the 4
    # queues by their assigned sem index.
    import re
    for b in nc.main_func.blocks:
        for ins in b.instructions:
            if type(ins).__name__ == "InstDMAGatherAnt" and ins.sync_info:
                for upd in ins.sync_info.on_update:
                    mname = upd.ant_name or ""
                    m = re.match(r"DMASW(\d+)_", mname)
                    if m:
                        ins.queue_num = int(m.group(1)) % 4
                        break

    nc.compile()
    return nc


# -------------------------------------------------------------------- entry
_CACHE = {}


def kernel(atom_feats, bond_feats, W_i, W_h, W_o, src, dst, reverse_e):
    from concourse import bass_utils

    NPAD, NWIN, OUT_SLICE = _derived()

    rev = np.asarray(reverse_e).astype(np.int64)
    ar = np.arange(N_PAIRS, dtype=np.int64)
    assert np.array_equal(rev[:N_PAIRS], ar + N_PAIRS) and \
        np.array_equal(rev[N_PAIRS:], ar), "unexpected reverse_e structure"

    meta, in_maps = _host_prep(atom_feats, bond_feats, W_i, W_h, W_o, src, dst)

    key = (meta["S_TOT"], meta["C"], meta["F00"], meta["F11"], meta["S01"])
    if key not in _CACHE:
        _CACHE[key] = _build_program(meta)
    nc = _CACHE[key]

    res = bass_utils.run_bass_kernel_spmd(
        nc, in_maps, core_ids=list(range(N_CORES)))
    out = np.concatenate(
        [res.results[c]["out"].T for c in range(N_CORES)], axis=0)
    return np.ascontiguousarray(out[:N_NODES]).astype(np.float32)



# revision 24
# speedup vs baseline: 1.8337x; 1.0677x over previous
"""D-MPNN encoder layer on 8 Trainium2 NeuronCores (Bass/Tile), v2.

Sharding strategy
-----------------
Edge pairs are dealt across 8 cores balanced by (class, dst-window) bucket;
node space split into two halves (NH=25088) so gather tables fit int16.
Edges grouped into 4 classes by (src-half, dst-half); WITHIN each class,
edges are sorted by dst window (128 nodes per window) with per-window slot
counts uniformized across cores (max over cores, min 1) so the program is
SPMD-static while per-core data fills real edges first + pad.

Per message layer (fp16 storage, fp32 PSUM):
  m = dma_gather(tmp_half[src]) - dma_gather(h_prev[rev])   (both random,
      ~600M rows/s on this HW, measured)
  h = relu(W_h @ m) via PE transpose per 128-edge chunk + matmuls
  segment-sum FUSED: as h chunks appear (already dst-window-sorted), DVE
  one-hot selection matmuls accumulate each window's 128-node partial in
  PSUM; windows finalize to a per-half SBUF buffer (class A) or add+DMA to
  partials (class B). No re-gather of h for the segment sum.
Collectives are split per node half (AllReduce layers 0,1; ReduceScatter
layer 2) and overlap the other half's message stage / next layer.
Layer 0 gathers proj = atom @ W_i_atom.T (built per half, overlapped) and
adds a streaming W_i_bond matmul. Final layer: dense per-node matmul on
each core's scattered node slice; host assembles.
"""

import sys
import numpy as np

sys.path.insert(0, "/opt/trn_rl_repo")

# ---------------------------------------------------------------- constants
N_NODES = 50000
N_PAIRS = 400000
ATOM_FDIM = 133
BOND_FDIM = 14
HIDDEN = 128
DEPTH = 3
N_CORES = 8
NH = 25088                      # node half size (196 windows of 128)
NWIN = NH // 128
NPAD = 2 * NH
RS_SLICE = NH // N_CORES        # 3136 rows per core per half
OUT_SLICE = 2 * RS_SLICE

GOP = 4096                      # rows per dma_gather instruction
BLK = 512                       # matmul free-dim block

F16 = np.float16
I16 = np.int16

CLS_NAMES = ["00", "01", "10", "11"]
PAIR = {"00": "00", "01": "10", "10": "01", "11": "11"}
DEBUG_DUMPS = []
ABLATE = frozenset()  # {"noseg","norev","nosrc","nowh","nocoll","nohw","noproj"}


def _wrap_idx(idx):
    """int16 index array -> dma_gather SBUF layout [128, n/16]."""
    n = idx.shape[0]
    assert n % 16 == 0
    return np.tile(idx.reshape(n // 16, 16).T, (8, 1)).copy()


def _ceil_to(x, m):
    return ((x + m - 1) // m) * m


def _host_prep(atom_feats, bond_feats, W_i, W_h, W_o, src, dst):
    src = np.asarray(src).astype(np.int64)
    dst = np.asarray(dst).astype(np.int64)
    E = 2 * N_PAIRS

    # directed edges: e < N_PAIRS forward, e >= N_PAIRS reverse (rev = e^P)
    cls_id = 2 * (src >= NH).astype(np.int64) + (dst >= NH).astype(np.int64)
    dstl_all = dst - (dst >= NH) * NH
    srcl_all = src - (src >= NH) * NH
    win_all = dstl_all // 128

    # ---- balanced dealing of pairs to cores by (fwd bucket, rev bucket)
    fwdb = cls_id[:N_PAIRS] * NWIN + win_all[:N_PAIRS]
    revb = cls_id[N_PAIRS:] * NWIN + win_all[N_PAIRS:]
    order = np.lexsort((revb, fwdb))
    core_of_pair = np.empty(N_PAIRS, dtype=np.int64)
    core_of_pair[order] = np.arange(N_PAIRS) % N_CORES
    core_of_edge = np.concatenate([core_of_pair, core_of_pair])

    # ---- per (core, class): edge lists sorted by dst_local
    # counts per (core, class, window) -> uniform L = max over cores (>=1)
    edge_core_cls = core_of_edge * 4 + cls_id
    cnt = np.zeros((N_CORES, 4, NWIN), dtype=np.int64)
    np.add.at(cnt.reshape(-1), edge_core_cls * NWIN + win_all,
              np.ones(E, dtype=np.int64))
    L = np.maximum(cnt.max(axis=0), 1)            # [4, NWIN]
    start = np.zeros((4, NWIN + 1), dtype=np.int64)
    np.cumsum(L, axis=1, out=start[:, 1:])
    SZ0 = start[:, -1].copy()                     # real+win-pad slots
    SZ = np.array([_ceil_to(int(s), 512) for s in SZ0])
    assert SZ.max() < 32768, f"class table too big for int16: {SZ}"
    cls_off = np.zeros(5, dtype=np.int64)
    np.cumsum(SZ, out=cls_off[1:])
    S_TOTAL = int(cls_off[-1])

    # ---- static (block, window) pair structure per class
    bw_pairs = []                                 # per class: list of (b, w)
    for ci in range(4):
        pairs = []
        for w in range(NWIN):
            b0 = int(start[ci, w]) // 128
            b1 = (int(start[ci, w + 1]) - 1) // 128
            for b in range(b0, b1 + 1):
                pairs.append((b, w))
        pairs.sort(key=lambda t: (t[0], t[1]))
        bw_pairs.append(pairs)
    col_base = [0]
    for ci in range(4):
        col_base.append(col_base[-1] + len(bw_pairs[ci]))
    NCOL = _ceil_to(col_base[-1], 4)

    meta = dict(SZ=tuple(int(x) for x in SZ),
                cls_off=tuple(int(x) for x in cls_off),
                S_TOTAL=S_TOTAL,
                bw_pairs=tuple(tuple(p) for p in bw_pairs),
                col_base=tuple(col_base), NCOL=NCOL)

    # ---- per-core slot data
    bond_feats = np.asarray(bond_feats, dtype=np.float32)
    atom_pad = np.zeros((NPAD, ATOM_FDIM), dtype=np.float32)
    atom_pad[:N_NODES] = np.asarray(atom_feats, dtype=np.float32)
    atomT = np.ascontiguousarray(atom_pad.T).astype(F16)

    W_i = np.asarray(W_i, dtype=np.float32)
    W_h = np.asarray(W_h, dtype=np.float32)
    W_o = np.asarray(W_o, dtype=np.float32)
    wiaT = np.ascontiguousarray(W_i[:, :ATOM_FDIM].T).astype(F16)
    wibT = np.ascontiguousarray(W_i[:, ATOM_FDIM:].T).astype(F16)
    whT = np.ascontiguousarray(W_h.T).astype(F16)
    woaT = np.ascontiguousarray(W_o[:, :ATOM_FDIM].T).astype(F16)
    womT = np.ascontiguousarray(W_o[:, ATOM_FDIM:].T).astype(F16)

    iotaf = np.tile(np.arange(128, dtype=F16)[None, :], (128, 8)).copy()
    ident = np.eye(128, dtype=F16)

    shared = {
        "atomT_a": atomT[:128], "atomT_b": atomT[128:ATOM_FDIM],
        "wia_a": wiaT[:128], "wia_b": wiaT[128:ATOM_FDIM], "wib": wibT,
        "wh": whT, "woa_a": woaT[:128], "woa_b": woaT[128:ATOM_FDIM],
        "wom": womT, "iotaf": iotaf, "ident": ident,
    }

    eid = np.arange(E, dtype=np.int64)
    rev_eid = (eid + N_PAIRS) % E

    in_maps = []
    for c in range(N_CORES):
        srcl16 = np.zeros(S_TOTAL, dtype=I16)
        rev16 = np.zeros(S_TOTAL, dtype=I16)
        gid = np.full(S_TOTAL, -1, dtype=np.int64)
        dloc = np.full(S_TOTAL, 10 ** 6, dtype=np.int64)  # pad sentinel

        slot_of_eid = np.full(E, -1, dtype=np.int64)
        for ci in range(4):
            m = (core_of_edge == c) & (cls_id == ci)
            es = eid[m]
            # sort by dst window, then src_local (gather locality within run)
            o = np.lexsort((es, srcl_all[es], win_all[es]))
            es = es[o]
            # slots: window-run layout, real edges at run start
            w_of = win_all[es]
            # es sorted by dstl -> already grouped by window ascending
            cnts = np.bincount(w_of, minlength=NWIN)
            slots = np.empty(es.shape[0], dtype=np.int64)
            pos = 0
            for w in range(NWIN):
                k = int(cnts[w])
                s0 = int(start[ci, w])
                slots[pos:pos + k] = s0 + np.arange(k)
                pos += k
            assert pos == es.shape[0]
            gslots = cls_off[ci] + slots
            srcl16[gslots] = srcl_all[es].astype(I16)
            gid[gslots] = es
            dloc[gslots] = dstl_all[es]
            slot_of_eid[es] = gslots

        # rev links (both edges of a pair are on the same core)
        real = gid >= 0
        rg = gid[real]
        rs = slot_of_eid[rev_eid[rg]]
        ci_of_slot = np.searchsorted(cls_off[1:], np.nonzero(real)[0],
                                     side="right")
        rev_cls = np.array([CLS_NAMES.index(PAIR[CLS_NAMES[i]])
                            for i in range(4)])[ci_of_slot]
        rev16[real] = (rs - cls_off[rev_cls]).astype(I16)

        # bond features in slot order
        bondT = np.zeros((BOND_FDIM, S_TOTAL), dtype=F16)
        bondT[:, real] = bond_feats[rg].T.astype(F16)

        # one-hot dstl columns
        dstl = np.full((128, NCOL), 200.0, dtype=F16)
        for ci in range(4):
            for j, (b, w) in enumerate(bw_pairs[ci]):
                col = col_base[ci] + j
                sl = cls_off[ci] + 128 * b + np.arange(128)
                dv = dloc[sl] - 128 * w
                vals = np.where((dv >= 0) & (dv < 128), dv, 200)
                dstl[:, col] = vals.astype(F16)

        sl0 = slice(c * RS_SLICE, (c + 1) * RS_SLICE)
        sl1 = slice(NH + c * RS_SLICE, NH + (c + 1) * RS_SLICE)
        atomS = np.concatenate([atomT[:, sl0], atomT[:, sl1]], axis=1)

        m = dict(shared)
        m["bondT"] = bondT
        m["src16w"] = _wrap_idx(srcl16)
        m["rev16w"] = _wrap_idx(rev16)
        m["dstl"] = dstl
        m["atomS_a"] = np.ascontiguousarray(atomS[:128])
        m["atomS_b"] = np.ascontiguousarray(atomS[128:ATOM_FDIM])
        in_maps.append(m)

    return meta, in_maps


# ------------------------------------------------------------------ program
def _build_program(meta):
    import concourse.bacc as bacc
    import concourse.tile as tile
    import concourse.mybir as mybir
    from concourse import library_config

    f16, f32, i16 = mybir.dt.float16, mybir.dt.float32, mybir.dt.int16
    Relu = mybir.ActivationFunctionType.Relu

    SZ = meta["SZ"]
    cls_off = meta["cls_off"]
    S_TOTAL = meta["S_TOTAL"]
    bw_pairs = meta["bw_pairs"]
    col_base = meta["col_base"]
    NCOL = meta["NCOL"]

    nc = bacc.Bacc("TRN2", target_bir_lowering=False, debug=False,
                   enable_asserts=False, num_devices=N_CORES,
                   num_swdge_queues=4)

    def din(name, shape, dt=f16):
        return nc.dram_tensor(name, shape, dt, kind="ExternalInput").ap()

    atomT_a = din("atomT_a", [128, NPAD])
    atomT_b = din("atomT_b", [ATOM_FDIM - 128, NPAD])
    atomS_a = din("atomS_a", [128, OUT_SLICE])
    atomS_b = din("atomS_b", [ATOM_FDIM - 128, OUT_SLICE])
    wia_a = din("wia_a", [128, HIDDEN])
    wia_b = din("wia_b", [ATOM_FDIM - 128, HIDDEN])
    wib = din("wib", [BOND_FDIM, HIDDEN])
    wh_t = din("wh", [HIDDEN, HIDDEN])
    woa_a = din("woa_a", [128, HIDDEN])
    woa_b = din("woa_b", [ATOM_FDIM - 128, HIDDEN])
    wom = din("wom", [HIDDEN, HIDDEN])
    iotaf = din("iotaf", [128, 1024])
    ident_t = din("ident", [128, 128])
    bondT = din("bondT", [BOND_FDIM, S_TOTAL])
    src16w = din("src16w", [128, S_TOTAL // 16], i16)
    rev16w = din("rev16w", [128, S_TOTAL // 16], i16)
    dstl_t = din("dstl", [128, NCOL])

    out_t = nc.dram_tensor("out", [HIDDEN, OUT_SLICE], f32,
                           kind="ExternalOutput").ap()

    proj = [nc.dram_tensor(f"proj{d}", [NH, HIDDEN], f16,
                           kind="Internal").ap() for d in (0, 1)]
    h_cls = {}
    for ell in range(DEPTH - 1):
        for cn in CLS_NAMES:
            h_cls[(ell, cn)] = nc.dram_tensor(
                f"h{ell}_{cn}", [SZ[CLS_NAMES.index(cn)], HIDDEN], f16,
                kind="Internal").ap()
    partials = {}
    tmp = {}
    for ell in range(DEPTH):
        for d in (0, 1):
            partials[(ell, d)] = nc.dram_tensor(
                f"partials{ell}_{d}", [NH, HIDDEN], f16,
                kind="Internal").ap()
            if ell < DEPTH - 1:
                tmp[(ell, d)] = nc.dram_tensor(
                    f"tmp{ell}_{d}", [NH, HIDDEN], f16, kind="Internal",
                    addr_space="Shared").ap()
    rs_out = [nc.dram_tensor(f"rsout{d}", [RS_SLICE, HIDDEN], f16,
                             kind="Internal").ap() for d in (0, 1)]

    nc.gpsimd.load_library(library_config.mlp)

    with tile.TileContext(nc) as tc:
        with (
            tc.tile_pool(name="pers", bufs=1) as pers,
            tc.tile_pool(name="shalf", bufs=1) as shalf,
            tc.tile_pool(name="work", bufs=2) as work,
            tc.tile_pool(name="segw", bufs=2) as segw,
            tc.tile_pool(name="psum", bufs=2, space="PSUM") as psum,
            tc.tile_pool(name="psum1", bufs=2, space="PSUM") as psum1,
            tc.tile_pool(name="psum2", bufs=4, space="PSUM") as psum2,
        ):
            # ---------- persistent SBUF
            def pload(ap_in, shape, tag, dt=f16):
                t = pers.tile(shape, dt, tag=tag)
                nc.sync.dma_start(t[:], ap_in)
                return t

            w_wh = pload(wh_t[:], [HIDDEN, HIDDEN], "w_wh")
            w_wib = pload(wib[:], [BOND_FDIM, HIDDEN], "w_wib")
            w_wia_a = pload(wia_a[:], [128, HIDDEN], "w_wia_a")
            w_wia_b = pload(wia_b[:], [ATOM_FDIM - 128, HIDDEN], "w_wia_b")
            w_woa_a = pload(woa_a[:], [128, HIDDEN], "w_woa_a")
            w_woa_b = pload(woa_b[:], [ATOM_FDIM - 128, HIDDEN], "w_woa_b")
            w_wom = pload(wom[:], [HIDDEN, HIDDEN], "w_wom")
            io_t = pload(iotaf[:], [128, 1024], "io_t")
            sidx = pload(src16w[:], [128, S_TOTAL // 16], "sidx", i16)
            ridx = pload(rev16w[:], [128, S_TOTAL // 16], "ridx", i16)
            dstl_s = pload(dstl_t[:], [128, NCOL], "dstl_s")
            ident_s = pload(ident_t[:], [128, 128], "ident_s")

            # ---------- proj per half [NH, H] f16
            for d in (() if "noproj" in ABLATE else (0, 1)):
                for ch in range(NWIN):
                    a_t = work.tile([128, 128], f16, tag="pa")
                    b_t = work.tile([ATOM_FDIM - 128, 128], f16, tag="pb")
                    csl = slice(d * NH + ch * 128, d * NH + (ch + 1) * 128)
                    nc.sync.dma_start(a_t[:], atomT_a[:, csl])
                    nc.sync.dma_start(b_t[:], atomT_b[:, csl])
                    ps = psum1.tile([128, 128], f32, tag="seg")
                    nc.tensor.matmul(ps[:], lhsT=a_t[:], rhs=w_wia_a[:],
                                     start=True, stop=False)
                    nc.tensor.matmul(ps[:], lhsT=b_t[:], rhs=w_wia_b[:],
                                     start=False, stop=True)
                    o_t = work.tile([128, 128], f16, tag="po")
                    nc.scalar.copy(o_t[:], ps[:])
                    nc.sync.dma_start(
                        proj[d][ch * 128:(ch + 1) * 128, :].rearrange(
                            "(a p) d -> p a d", p=128),
                        o_t[:].unsqueeze(1))

            # ---------- layers: class order 00,10 (half0) then 01,11 (half1)
            def m_stage(ell, cn, second):
                ci = CLS_NAMES.index(cn)
                s_half, d_half = int(cn[0]), int(cn[1])
                o, sz = cls_off[ci], SZ[ci]
                pairs = bw_pairs[ci]
                if ell == 0:
                    table = proj[s_half]
                else:
                    table = tmp[(ell - 1, s_half)]
                if second:
                    S_t = m_stage.s_tiles[d_half]
                else:
                    S_t = shalf.tile([128, NWIN * 128], f16, tag="S")
                    m_stage.s_tiles[d_half] = S_t

                pi = 0                      # next bw pair index
                np_pairs = len(pairs)
                oh_tiles = {}               # col//8 -> tile
                ps_w, cur_w = None, -1
                pos = 0
                while pos < sz:
                    g = min(GOP, sz - pos)
                    nch = g // 128

                    # pre-generate this chunk's one-hot batches (8 cols each)
                    # on DVE so they're ready when PE reaches the seg matmuls
                    if "noseg" not in ABLATE:
                        b_hi0 = (pos + g) // 128
                        pj = pi
                        while pj < np_pairs and pairs[pj][0] < b_hi0:
                            col = col_base[ci] + pj
                            cb8 = (col // 8) * 8
                            if cb8 not in oh_tiles:
                                t8 = segw.tile([128, 1024], f16, tag="oh",
                                               bufs=10)
                                n8 = min(8, NCOL - cb8)
                                nc.vector.tensor_tensor(
                                    out=t8[:, :n8 * 128].rearrange(
                                        "p (c n) -> p c n", n=128),
                                    in0=io_t[:, :n8 * 128].rearrange(
                                        "p (c n) -> p c n", n=128),
                                    in1=dstl_s[:, cb8:cb8 + n8]
                                    .to_broadcast([128, n8, 128]),
                                    op=mybir.AluOpType.is_equal)
                                oh_tiles[cb8] = t8
                            pj += 1

                    icols = sidx[:, (o + pos) // 16:(o + pos + g) // 16]
                    g1 = work.tile([128, (GOP // 128) * HIDDEN], f16,
                                   tag="g1", bufs=3)
                    if "nosrc" not in ABLATE:
                        nc.gpsimd.dma_gather(
                            g1[:, :nch * HIDDEN].rearrange(
                                "p (c d) -> p c d", d=HIDDEN),
                            table[:], icols, g, g, HIDDEN,
                            single_packet=False)
                    h_t = work.tile([128, (GOP // 128) * HIDDEN], f16,
                                    tag="ht", bufs=3)
                    if ell == 0:
                        bt = work.tile([BOND_FDIM, GOP], f16, tag="bt")
                        nc.sync.dma_start(bt[:, :g],
                                          bondT[:, o + pos:o + pos + g])
                    elif "norev" not in ABLATE:
                        rcols = ridx[:, (o + pos) // 16:(o + pos + g) // 16]
                        g2 = work.tile([128, (GOP // 128) * HIDDEN], f16,
                                       tag="g2", bufs=3)
                        nc.gpsimd.dma_gather(
                            g2[:, :nch * HIDDEN].rearrange(
                                "p (c d) -> p c d", d=HIDDEN),
                            h_cls[(ell - 1, PAIR[cn])][:], rcols, g, g,
                            HIDDEN, single_packet=False)
                        nc.vector.tensor_tensor(
                            out=g1[:, :nch * HIDDEN],
                            in0=g1[:, :nch * HIDDEN],
                            in1=g2[:, :nch * HIDDEN],
                            op=mybir.AluOpType.subtract)
                    for j in range(g // BLK):
                        ps = psum.tile([128, BLK], f32, tag="mm")
                        if ell == 0:
                            for q in range(4):
                                cb = j * 4 + q
                                nc.tensor.matmul(
                                    ps[:, q * 128:(q + 1) * 128],
                                    lhsT=bt[:, cb * 128:(cb + 1) * 128],
                                    rhs=w_wib[:], start=True, stop=True)
                            sl = slice(j * BLK, (j + 1) * BLK)
                            nc.vector.tensor_add(out=h_t[:, sl],
                                                 in0=g1[:, sl], in1=ps[:])
                            nc.scalar.activation(h_t[:, sl], h_t[:, sl],
                                                 Relu)
                        elif "nowh" in ABLATE:
                            sl = slice(j * BLK, (j + 1) * BLK)
                            nc.scalar.activation(h_t[:, sl], g1[:, sl], Relu)
                        else:
                            # staged: 4 transposes, then 4 copies, then 4
                            # matmuls — PE never waits on the copy of the
                            # block it is about to multiply
                            mt = work.tile([128, BLK], f16, tag="mt")
                            tps = []
                            for q in range(4):
                                cb = j * 4 + q
                                tp = psum2.tile([128, 128], f16, tag="tp")
                                nc.tensor.transpose(
                                    tp[:], g1[:, cb * 128:(cb + 1) * 128],
                                    ident_s[:])
                                tps.append(tp)
                            for q in range(4):
                                msl = slice(q * 128, (q + 1) * 128)
                                nc.scalar.copy(mt[:, msl], tps[q][:])
                            for q in range(4):
                                msl = slice(q * 128, (q + 1) * 128)
                                nc.tensor.matmul(
                                    ps[:, msl], lhsT=mt[:, msl],
                                    rhs=w_wh[:], start=True, stop=True)
                            nc.scalar.activation(
                                h_t[:, j * BLK:(j + 1) * BLK], ps[:], Relu)
                    if ell < DEPTH - 1 and "nohw" not in ABLATE:
                        nc.sync.dma_start(
                            h_cls[(ell, cn)][pos:pos + g, :].rearrange(
                                "(c p) d -> p c d", p=128),
                            h_t[:, :nch * HIDDEN].rearrange(
                                "p (c d) -> p c d", d=HIDDEN))

                    # fused segment sum over blocks in this chunk
                    b_lo, b_hi = pos // 128, (pos + g) // 128
                    while "noseg" not in ABLATE and pi < np_pairs \
                            and pairs[pi][0] < b_hi:
                        b, w = pairs[pi]
                        col = col_base[ci] + pi
                        cb8 = (col // 8) * 8
                        oh_tile = oh_tiles[cb8]
                        first = (pi == 0) or (pairs[pi - 1][1] != w)
                        last = (pi == np_pairs - 1) or (pairs[pi + 1][1] != w)
                        if first:
                            ps_w = psum1.tile([128, HIDDEN], f32, tag="seg")
                            cur_w = w
                        assert cur_w == w
                        hb = (b - pos // 128) * 128
                        nc.tensor.matmul(
                            ps_w[:],
                            lhsT=oh_tile[:, (col - cb8) * 128:
                                         (col - cb8 + 1) * 128],
                            rhs=h_t[:, hb:hb + HIDDEN],
                            start=first, stop=last)
                        if last:
                            wsl = slice(w * 128, (w + 1) * 128)
                            if not second:
                                nc.scalar.copy(S_t[:, wsl], ps_w[:])
                            else:
                                p_t = segw.tile([128, HIDDEN], f16, tag="pt")
                                nc.vector.tensor_add(out=p_t[:], in0=ps_w[:],
                                                     in1=S_t[:, wsl])
                                nc.sync.dma_start(
                                    partials[(ell, d_half)]
                                    [w * 128:(w + 1) * 128, :].rearrange(
                                        "(a p) d -> p a d", p=128),
                                    p_t[:].unsqueeze(1))
                        pi += 1
                    pos += g
                assert "noseg" in ABLATE or pi == np_pairs

            m_stage.s_tiles = {}

            for ell in range(DEPTH):
                for d, (cnA, cnB) in ((0, ("00", "10")), (1, ("01", "11"))):
                    m_stage(ell, cnA, second=False)
                    m_stage(ell, cnB, second=True)
                    if "nocoll" in ABLATE:
                        pass
                    elif ell < DEPTH - 1:
                        nc.gpsimd.collective_compute(
                            "AllReduce", mybir.AluOpType.add,
                            replica_groups=[list(range(N_CORES))],
                            ins=[partials[(ell, d)][:]],
                            outs=[tmp[(ell, d)][:]])
                    else:
                        nc.gpsimd.collective_compute(
                            "ReduceScatter", mybir.AluOpType.add,
                            replica_groups=[list(range(N_CORES))],
                            ins=[partials[(ell, d)][:]],
                            outs=[rs_out[d][:]])

            # ---------- final: out.T = relu(WoA@atom.T + WoM@msg.T)
            for ch in range(OUT_SLICE // 128):
                csl = slice(ch * 128, (ch + 1) * 128)
                a_t = work.tile([128, 128], f16, tag="fa")
                b_t = work.tile([ATOM_FDIM - 128, 128], f16, tag="fb")
                m_t = work.tile([128, 128], f16, tag="fm")
                mraw = work.tile([128, 128], f16, tag="fmr")
                nc.sync.dma_start(a_t[:], atomS_a[:, csl])
                nc.sync.dma_start(b_t[:], atomS_b[:, csl])
                r0 = ch * 128
                if r0 + 128 <= RS_SLICE:
                    nc.sync.dma_start(mraw[:], rs_out[0][r0:r0 + 128, :])
                elif r0 >= RS_SLICE:
                    nc.sync.dma_start(
                        mraw[:], rs_out[1][r0 - RS_SLICE:
                                           r0 - RS_SLICE + 128, :])
                else:
                    k = RS_SLICE - r0
                    nc.sync.dma_start(mraw[:k, :], rs_out[0][r0:RS_SLICE, :])
                    nc.sync.dma_start(mraw[k:, :], rs_out[1][:128 - k, :])
                tpf = psum2.tile([128, 128], f16, tag="tp")
                nc.tensor.transpose(tpf[:], mraw[:], ident_s[:])
                nc.scalar.copy(m_t[:], tpf[:])
                ps = psum1.tile([128, 128], f32, tag="seg")
                nc.tensor.matmul(ps[:], lhsT=w_woa_a[:], rhs=a_t[:],
                                 start=True, stop=False)
                nc.tensor.matmul(ps[:], lhsT=w_woa_b[:], rhs=b_t[:],
                                 start=False, stop=False)
                nc.tensor.matmul(ps[:], lhsT=w_wom[:], rhs=m_t[:],
                                 start=False, stop=True)
                o_t = work.tile([128, 128], f32, tag="fo")
                nc.scalar.activation(o_t[:], ps[:], Relu)
                nc.sync.dma_start(out_t[:, csl], o_t[:])

    if DEBUG_DUMPS:
        with tile.TileContext(nc) as tc2:
            dbg_map = {"proj0": proj[0], "proj1": proj[1],
                       "rs0": rs_out[0], "rs1": rs_out[1]}
            for ell in range(DEPTH - 1):
                for cn in CLS_NAMES:
                    dbg_map[f"h{ell}_{cn}"] = h_cls[(ell, cn)]
            for ell in range(DEPTH):
                for d in (0, 1):
                    dbg_map[f"partials{ell}_{d}"] = partials[(ell, d)]
                    if ell < DEPTH - 1:
                        dbg_map[f"tmp{ell}_{d}"] = tmp[(ell, d)]
            for name in DEBUG_DUMPS:
                t = dbg_map[name]
                od = nc.dram_tensor("dbg_" + name, list(t.shape), f16,
                                    kind="ExternalOutput").ap()
                nc.sync.dma_start(od[:], t[:])

    # Tile assigns SWDGE completion sems round-robin (DMASW<i>_*); the HW
    # locks each sem to one SWDGE queue, so spread gathers across the 4
    # queues by their assigned sem index.
    import re
    for b in nc.main_func.blocks:
        for ins in b.instructions:
            if type(ins).__name__ == "InstDMAGatherAnt" and ins.sync_info:
                for upd in ins.sync_info.on_update:
                    mname = upd.ant_name or ""
                    m = re.match(r"DMASW(\d+)_", mname)
                    if m:
                        ins.queue_num = int(m.group(1)) % 4
                        break

    nc.compile()
    return nc


# -------------------------------------------------------------------- entry
_CACHE = {}


def kernel(atom_feats, bond_feats, W_i, W_h, W_o, src, dst, reverse_e):
    from concourse import bass_utils

    rev = np.asarray(reverse_e).astype(np.int64)
    ar = np.arange(N_PAIRS, dtype=np.int64)
    assert np.array_equal(rev[:N_PAIRS], ar + N_PAIRS) and \
        np.array_equal(rev[N_PAIRS:], ar), "unexpected reverse_e structure"

    meta, in_maps = _host_prep(atom_feats, bond_feats, W_i, W_h, W_o,
                               src, dst)

    key = (meta["SZ"], meta["bw_pairs"])
    if key not in _CACHE:
        _CACHE[key] = _build_program(meta)
    nc = _CACHE[key]

    res = bass_utils.run_bass_kernel_spmd(
        nc, in_maps, core_ids=list(range(N_CORES)))
    full = np.empty((NPAD, HIDDEN), dtype=np.float32)
    for c in range(N_CORES):
        o = res.results[c]["out"]
        full[c * RS_SLICE:(c + 1) * RS_SLICE] = o[:, :RS_SLICE].T
        full[NH + c * RS_SLICE:NH + (c + 1) * RS_SLICE] = o[:, RS_SLICE:].T
    return np.ascontiguousarray(full[:N_NODES]).astype(np.float32)
